# revision 14
# baseline (speedup 1.0000x reference)
"""Trainium2 Bass kernel for the MAB dense-transformer block.

Sharding: 8 cores = 2 batches x 4 Sq-slices (512 each). The K/V projection
is additionally sharded across the 4 cores of each batch (core c computes
k/v only for head-group c%4) and exchanged with two in-group AllGathers,
eliminating the 4x replication of the baseline.

Attention per core: 16 heads x 512 q-cols, processed in two 256-col halves
so the second half's ACT(sigmoid) window overlaps the first half's
proj+FFN matmuls on the PE.
  - logits^T chunks [128k x 256q] per head, E/O head pairs run as
    concurrent 64-contraction row-groups,
  - sigmoid: one 2048-wide ACT per 512 k-positions covering both heads
    (cols = [u0E u0O u1E u1O u2E u2O u3E u3O] x 256), output fp8,
  - o^T accumulated with fp8 DoubleRow matmuls (contraction 256) with a
    ones-column in vT giving row-sums for the renormalization for free,
  - renorm: DVE reciprocal of the sums row + gpsimd partition-broadcast.
Projections / FFN in bf16 with fp32 PSUM accumulation.
"""

import numpy as np
import ml_dtypes

BF = ml_dtypes.bfloat16
F8 = ml_dtypes.float8_e4m3fn

B, DIM, H, DK, SQ, SK = 2, 1024, 16, 64, 2048, 2048
D = H * DK
NCORES = 8
QSL = SQ // 4          # 512 columns of Sq per core
W = 256                # attention/FFN column-half width
NG = 4                 # head groups (4 heads each); also kv-shard count
GH = H // NG

_nc_cache = {}
_host_cache = {}


def _build_nc(mask_ones, bq_nz, bk_nz, bp_nz, b2_nz):
    from concourse import bacc, mybir
    import concourse.tile as tile

    bf16 = mybir.dt.bfloat16
    f32 = mybir.dt.float32
    fp8 = mybir.dt.float8e4
    AF = mybir.ActivationFunctionType
    DR = mybir.MatmulPerfMode.DoubleRow

    nc = bacc.Bacc("TRN2", num_devices=NCORES)

    d_Kb = nc.declare_dram_parameter("Kb", [DIM, SK], bf16, isOutput=False)
    d_Qb = nc.declare_dram_parameter("Qb", [DIM, QSL], bf16, isOutput=False)
    d_Qres = nc.declare_dram_parameter("Qres", [DIM, QSL], f32, isOutput=False)
    d_wq = nc.declare_dram_parameter("wq", [8, 128, 8, 128], bf16, isOutput=False)
    d_wk = nc.declare_dram_parameter("wk", [2, 128, 8, 128], bf16, isOutput=False)
    d_wv = nc.declare_dram_parameter("wv", [8, 128, GH * DK], bf16, isOutput=False)
    d_wp = nc.declare_dram_parameter("wp", [8, 128, 8, 128], bf16, isOutput=False)
    d_w1 = nc.declare_dram_parameter("w1", [16, 128, 8, 128], bf16, isOutput=False)
    d_w2 = nc.declare_dram_parameter("w2", [8, 128, 16, 128], bf16, isOutput=False)
    d_b1 = nc.declare_dram_parameter("b1t", [128, 16], f32, isOutput=False)
    d_bq = d_bk = d_bp = d_b2 = d_madd = None
    if bq_nz:
        d_bq = nc.declare_dram_parameter("bqt", [128, 8], f32, isOutput=False)
    if bk_nz:
        d_bk = nc.declare_dram_parameter("bkt", [128, 2], f32, isOutput=False)
    if bp_nz:
        d_bp = nc.declare_dram_parameter("bpt", [128, 8], f32, isOutput=False)
    if b2_nz:
        d_b2 = nc.declare_dram_parameter("b2t", [128, 8], f32, isOutput=False)
    if not mask_ones:
        d_madd = nc.declare_dram_parameter("maddt", [128, 16], f32, isOutput=False)
    d_out = nc.declare_dram_parameter("out", [DIM, QSL], f32, isOutput=True)

    VTW = 80  # padded per-head vT row (64 dims + ones col + pad for DR stride)

    with tile.TileContext(nc) as tc:
        with (
            tc.tile_pool(name="pin", bufs=1) as pin,
            tc.tile_pool(name="pw", bufs=4) as pw,
            tc.tile_pool(name="pkv", bufs=1) as pkv,
            tc.tile_pool(name="pq", bufs=1) as pq,
            tc.tile_pool(name="pwt", bufs=3) as pwt,
            tc.tile_pool(name="po", bufs=1) as po,
            tc.tile_pool(name="ph", bufs=1) as ph,
            tc.tile_pool(name="psmall", bufs=3) as psmall,
            tc.tile_pool(name="pconst", bufs=1) as pconst,
            tc.tile_pool(name="pout", bufs=2) as pout,
            tc.tile_pool(name="ppsL", bufs=1, space="PSUM") as ppsL,
            tc.tile_pool(name="ppsO", bufs=2, space="PSUM") as ppsO,
            tc.tile_pool(name="ppsP", bufs=2, space="PSUM") as ppsP,
            tc.tile_pool(name="pdram", bufs=1, space="DRAM") as pdram,
        ):
            # ---- input loads ----
            kb = pin.tile([128, 8, SK], bf16, tag="kb")
            kbr = d_Kb[:].rearrange("(c p) s -> p c s", p=128)
            for c in range(8):
                nc.sync.dma_start(out=kb[:, c, :], in_=kbr[:, c, :])
            wv_sb = pin.tile([128, 8, GH * DK], bf16, tag="wv")
            for c in range(8):
                nc.sync.dma_start(out=wv_sb[:, c, :], in_=d_wv[c])
            qb = pin.tile([128, 8, QSL], bf16, tag="qb")
            qbr = d_Qb[:].rearrange("(c p) s -> p c s", p=128)
            for c in range(8):
                nc.sync.dma_start(out=qb[:, c, :], in_=qbr[:, c, :])

            b1_sb = pconst.tile([128, 16], f32, tag="b1")
            nc.sync.dma_start(out=b1_sb, in_=d_b1[:])
            bq_sb = bk_sb = bp_sb = b2_sb = madd_sb = None
            if bq_nz:
                bq_sb = pconst.tile([128, 8], f32, tag="bq")
                nc.sync.dma_start(out=bq_sb, in_=d_bq[:])
            if bk_nz:
                bk_sb = pconst.tile([128, 2], f32, tag="bk")
                nc.sync.dma_start(out=bk_sb, in_=d_bk[:])
            if bp_nz:
                bp_sb = pconst.tile([128, 8], f32, tag="bp")
                nc.sync.dma_start(out=bp_sb, in_=d_bp[:])
            if b2_nz:
                b2_sb = pconst.tile([128, 8], f32, tag="b2")
                nc.sync.dma_start(out=b2_sb, in_=d_b2[:])
            if not mask_ones:
                madd_sb = pconst.tile([128, 16], f32, tag="madd")
                nc.sync.dma_start(out=madd_sb, in_=d_madd[:])

            # ---- local k/v projection (own head-group only) ----
            kg_loc = pkv.tile([128, 2, SK], bf16, tag="kgl")
            for lm in range(2):
                wt = pw.tile([128, 8, 128], bf16, tag="w")
                nc.sync.dma_start(out=wt, in_=d_wk[lm])
                for n in range(4):
                    ps = ppsP.tile([128, 512], f32, tag="pp")
                    for c in range(8):
                        nc.tensor.matmul(
                            ps, wt[:, c, :], kb[:, c, 512 * n : 512 * n + 512],
                            start=(c == 0), stop=(c == 7),
                        )
                    if bk_nz:
                        nc.scalar.activation(
                            kg_loc[:, lm, 512 * n : 512 * n + 512], ps, AF.Identity,
                            bias=bk_sb[:, lm : lm + 1],
                        )
                    else:
                        nc.vector.tensor_copy(
                            kg_loc[:, lm, 512 * n : 512 * n + 512], ps
                        )

            vt_loc = pkv.tile([128, 8, 2, GH, VTW], fp8, tag="vtl")
            nc.vector.memset(vt_loc.rearrange("p tp ko h d -> p (tp ko h d)"), 1.0)
            for t in range(16):
                tp, ko = t // 2, t % 2
                ps = ppsP.tile([128, 512], f32, tag="pp")
                for c in range(8):
                    nc.tensor.matmul(
                        ps[:, 0 : GH * DK],
                        kb[:, c, 128 * t : 128 * t + 128],
                        wv_sb[:, c, :],
                        start=(c == 0), stop=(c == 7),
                    )
                nc.vector.tensor_copy(
                    vt_loc[:, tp, ko, :, 0:DK],
                    ps[:, 0 : GH * DK].rearrange("p (h d) -> p h d", h=GH),
                )

            # ---- AllGather k/v within the 4-core batch group ----
            cc_k_in = pdram.tile([2 * 128, SK], bf16, tag="cki")
            cc_v_in = pdram.tile([128, 8 * 2 * GH * VTW], fp8, tag="cvi")
            cc_k_out = pdram.tile([8 * 128, SK], bf16, tag="cko")
            cc_v_out = pdram.tile([4 * 128, 8 * 2 * GH * VTW], fp8, tag="cvo")
            nc.gpsimd.dma_start(
                out=cc_k_in[:].rearrange("(m p) s -> p m s", p=128), in_=kg_loc
            )
            nc.gpsimd.dma_start(
                out=cc_v_in[:],
                in_=vt_loc.rearrange("p tp ko h d -> p (tp ko h d)"),
            )
            rg = [[0, 1, 2, 3], [4, 5, 6, 7]]
            nc.gpsimd.collective_compute(
                "AllGather", mybir.AluOpType.bypass, replica_groups=rg,
                ins=[cc_k_in[:].opt()], outs=[cc_k_out[:].opt()],
            )
            nc.gpsimd.collective_compute(
                "AllGather", mybir.AluOpType.bypass, replica_groups=rg,
                ins=[cc_v_in[:].opt()], outs=[cc_v_out[:].opt()],
            )

            # ---- q projection (overlaps the collectives) ----
            q_sb = pq.tile([128, 8, QSL], bf16, tag="q")
            for m in range(8):
                wt = pw.tile([128, 8, 128], bf16, tag="w")
                nc.sync.dma_start(out=wt, in_=d_wq[m])
                ps = ppsP.tile([128, 512], f32, tag="pp")
                for c in range(8):
                    nc.tensor.matmul(
                        ps, wt[:, c, :], qb[:, c, :],
                        start=(c == 0), stop=(c == 7),
                    )
                if bq_nz:
                    nc.scalar.activation(
                        q_sb[:, m, :], ps, AF.Identity, bias=bq_sb[:, m : m + 1]
                    )
                else:
                    nc.vector.tensor_copy(q_sb[:, m, :], ps)

            # ---- gather k/v back into SBUF (full, all 16 heads) ----
            kg = pq.tile([128, 8, SK], bf16, tag="kg")
            ckr = cc_k_out[:].rearrange("(m p) s -> p m s", p=128)
            for m in range(8):
                nc.sync.dma_start(out=kg[:, m, :], in_=ckr[:, m, :])
            vt = pq.tile([128, 8, 2, H, VTW], fp8, tag="vt")
            for r in range(4):
                nc.sync.dma_start(
                    out=vt[:, :, :, GH * r : GH * r + GH, :],
                    in_=cc_v_out[128 * r : 128 * r + 128, :].rearrange(
                        "p (tp ko h d) -> p tp ko h d", tp=8, ko=2, h=GH
                    ),
                )

            # ---- resident wp weights (used in both halves) ----
            wp_sb = pin.tile([128, 8, 8, 128], bf16, tag="wp")
            for m in range(8):
                nc.sync.dma_start(out=wp_sb[:, m, :, :], in_=d_wp[m])

            o_sb = po.tile([128, 8, QSL], bf16, tag="o")
            o_ff = po.tile([128, 8, QSL], bf16, tag="off")

            def emit_head_pair(g, j, hw):
                """Attention for heads (4g+2j, 4g+2j+1), q-cols [hw*W, hw*W+W).
                Per 512 k-positions: one [128,2048] logits psum (cols =
                [u0E u0O u1E u1O u2E u2O u3E u3O] x W), one 2048-wide sigmoid
                ACT into fp8, then 4 DoubleRow o-matmuls (trailing one block
                to keep the ACT latency off the PE path)."""
                oc = 2 * g + j
                ps_oEO = [
                    ppsO.tile([65, W], f32, tag="oacc", name="ps_oE"),
                    ppsO.tile([65, W], f32, tag="oacc", name="ps_oO"),
                ]

                def emit_omms(wts, kp):
                    for eo in range(2):
                        wr = wts[eo].rearrange("p (jj ko w) -> p jj ko w", jj=2, ko=2)
                        for jj in range(2):
                            nc.tensor.matmul(
                                ps_oEO[eo],
                                vt[:, 2 * kp + jj, :, 2 * oc + eo, 0:65],
                                wr[:, jj, :, :],
                                start=(kp == 0 and jj == 0),
                                stop=(kp == 3 and jj == 1),
                                perf_mode=DR,
                            )

                prev = None
                for kp in range(4):
                    ps_lE = ppsL.tile([128, 1024], f32, tag="lgE")
                    ps_lO = ppsL.tile([128, 1024], f32, tag="lgO")
                    for u in range(4):
                        t = 4 * kp + u
                        for eo, ps_l in ((0, ps_lE), (1, ps_lO)):
                            nc.tensor.matmul(
                                ps_l[:, u * W : (u + 1) * W],
                                kg[64 * eo : 64 * eo + 64, oc, 128 * t : 128 * t + 128],
                                q_sb[64 * eo : 64 * eo + 64, oc, hw * W : hw * W + W],
                                start=True, stop=True,
                            )
                    wtE = pwt.tile([128, 1024], fp8, tag="wtE")
                    wtO = pwt.tile([128, 1024], fp8, tag="wtO")
                    if mask_ones:
                        nc.scalar.activation(wtE, ps_lE, AF.Sigmoid)
                        nc.scalar.activation(wtO, ps_lO, AF.Sigmoid)
                    else:
                        for u in range(4):
                            t = 4 * kp + u
                            for wt_t, ps_l in ((wtE, ps_lE), (wtO, ps_lO)):
                                sl = slice(u * W, (u + 1) * W)
                                nc.scalar.activation(
                                    wt_t[:, sl], ps_l[:, sl], AF.Sigmoid,
                                    bias=madd_sb[:, t : t + 1],
                                )
                    if prev is not None:
                        emit_omms(*prev)
                    prev = ((wtE, wtO), kp)
                emit_omms(*prev)

                for eo in range(2):
                    sc = psmall.tile([1, W], f32, tag="sc")
                    nc.vector.tensor_copy(sc, ps_oEO[eo][64:65, :])
                    rc = psmall.tile([1, W], f32, tag="rc")
                    nc.vector.reciprocal_approx_fast(out=rc, in_=sc)
                    rb = psmall.tile([64, W], f32, tag="rb")
                    nc.gpsimd.partition_broadcast(rb, rc)
                    nc.vector.tensor_mul(
                        o_sb[64 * eo : 64 * eo + 64, oc, hw * W : hw * W + W],
                        ps_oEO[eo][0:64, :], rb,
                    )

            def emit_proj_ffn(hw, units_only=False):
                """wp + residual + FFN for q-cols [hw*W, hw*W+W). Returns a
                list of emission units so the caller can interleave them."""
                units = []
                cs = slice(hw * W, hw * W + W)

                def proj_unit(m):
                    ps = ppsP.tile([128, 512], f32, tag="pp")
                    for c in range(8):
                        nc.tensor.matmul(
                            ps[:, 0:W], wp_sb[:, m, c, :], o_sb[:, c, cs],
                            start=(c == 0), stop=(c == 7),
                        )
                    if bp_nz:
                        nc.scalar.activation(
                            ps[:, 0:W], ps[:, 0:W], AF.Identity,
                            bias=bp_sb[:, m : m + 1],
                        )
                    qr = psmall.tile([128, W], f32, tag="qr")
                    nc.sync.dma_start(
                        out=qr, in_=d_Qres[128 * m : 128 * m + 128, cs]
                    )
                    nc.vector.tensor_add(o_ff[:, m, cs], ps[:, 0:W], qr)

                h_sb = ph.tile([128, 16, W], bf16, tag="h", bufs=2)

                def ffn1_unit(mm):
                    wt = pw.tile([128, 8, 128], bf16, tag="w")
                    nc.sync.dma_start(out=wt, in_=d_w1[mm])
                    ps = ppsP.tile([128, 512], f32, tag="pp")
                    for c in range(8):
                        nc.tensor.matmul(
                            ps[:, 0:W], wt[:, c, :], o_ff[:, c, cs],
                            start=(c == 0), stop=(c == 7),
                        )
                    nc.scalar.activation(
                        h_sb[:, mm, :], ps[:, 0:W], AF.Relu, bias=b1_sb[:, mm : mm + 1]
                    )

                def ffn2_unit(m):
                    wt = pw.tile([128, 16, 128], bf16, tag="w2", bufs=2)
                    nc.sync.dma_start(out=wt, in_=d_w2[m])
                    ps = ppsP.tile([128, 512], f32, tag="pp")
                    for c in range(16):
                        nc.tensor.matmul(
                            ps[:, 0:W], wt[:, c, :], h_sb[:, c, :],
                            start=(c == 0), stop=(c == 15),
                        )
                    if b2_nz:
                        nc.scalar.activation(
                            ps[:, 0:W], ps[:, 0:W], AF.Identity,
                            bias=b2_sb[:, m : m + 1],
                        )
                    ot = pout.tile([128, W], f32, tag="out")
                    nc.vector.tensor_add(ot, ps[:, 0:W], o_ff[:, m, cs])
                    nc.sync.dma_start(
                        out=d_out[128 * m : 128 * m + 128, cs], in_=ot
                    )

                for m in range(0, 8, 2):
                    units.append(lambda m=m: (proj_unit(m), proj_unit(m + 1)))
                for mm in range(0, 16, 2):
                    units.append(lambda mm=mm: (ffn1_unit(mm), ffn1_unit(mm + 1)))
                for m in range(0, 8, 2):
                    units.append(lambda m=m: (ffn2_unit(m), ffn2_unit(m + 1)))
                return units

            # half 0: attention only (PE has spare time; nothing to fill yet)
            for g in range(NG):
                for j in range(2):
                    emit_head_pair(g, j, 0)
            # half 1: attention with half-0 proj+FFN interleaved into the
            # sigmoid-bound window (2 units per pair = 16 units total)
            units0 = emit_proj_ffn(0)
            ui = 0
            for g in range(NG):
                for j in range(2):
                    emit_head_pair(g, j, 1)
                    for _ in range(2):
                        if ui < len(units0):
                            units0[ui]()
                            ui += 1
            while ui < len(units0):
                units0[ui]()
                ui += 1
            for u in emit_proj_ffn(1):
                u()

    nc.finalize()
    return nc


def _tile_lhsT(wT, mt, ct):
    # wT [K, M] -> [M/128, 128, K/128, 128] tiles: [m, p, c, j] = wT[128c+p, 128m+j]
    K, M = wT.shape
    a = wT.reshape(K // 128, 128, M // 128, 128)
    return np.ascontiguousarray(a.transpose(2, 1, 0, 3))


def kernel(**inputs):
    np32 = lambda x: np.asarray(x, dtype=np.float32)
    Q = np32(inputs["Q"]); K = np32(inputs["K"]); mask = np32(inputs["mask"])
    wq = np32(inputs["wq"]); bq = np32(inputs["bq"])
    wk = np32(inputs["wk"]); bk = np32(inputs["bk"])
    wv = np32(inputs["wv"]); bv = np32(inputs["bv"])
    wp = np32(inputs["wp"]); bp = np32(inputs["bp"])
    w1 = np32(inputs["w1"]); b1 = np32(inputs["b1"])
    w2 = np32(inputs["w2"]); b2 = np32(inputs["b2"])

    scale = DK ** -0.5
    wq_eff = wq * scale
    bq_eff = bq * scale
    bp_eff = bp + wp @ bv          # fold v bias through the projection

    mask_ones = bool(np.all(mask == 1.0))
    bq_nz = bool(np.any(bq_eff)); bk_nz = bool(np.any(bk))
    bp_nz = bool(np.any(bp_eff)); b2_nz = bool(np.any(b2))

    key = (mask_ones, bq_nz, bk_nz, bp_nz, b2_nz)
    if key not in _nc_cache:
        _nc_cache[key] = _build_nc(*key)
    nc = _nc_cache[key]

    wkey = tuple(
        (a.__array_interface__["data"][0], a.shape)
        for a in (wq, wk, wv, wp, w1, w2, b1)
    )
    cached = _host_cache.get("w")
    if cached is not None and cached[0] == wkey:
        wq_t, wk_t, wv_t, wp_t, w1_t, w2_t, b1_t = cached[1]
    else:
        wq_t = _tile_lhsT(wq_eff.T, 8, 8).astype(BF)
        wk_t = _tile_lhsT(wk.T, 8, 8).astype(BF)
        wv_t = np.ascontiguousarray(wv.T.reshape(8, 128, D)).astype(BF)
        wp_t = _tile_lhsT(wp.T, 8, 8).astype(BF)
        w1_t = _tile_lhsT(w1.T, 16, 8).astype(BF)
        w2_t = _tile_lhsT(w2.T, 8, 16).astype(BF)
        b1_t = np.ascontiguousarray(b1.reshape(16, 128).T)
        _host_cache["w"] = (wkey, (wq_t, wk_t, wv_t, wp_t, w1_t, w2_t, b1_t))

    Kb_bf = [np.ascontiguousarray(K[b]).astype(BF) for b in range(B)]
    madd_t = [
        np.ascontiguousarray((-(1.0 - mask[b, 0]) * 10000.0).reshape(16, 128).T)
        for b in range(B)
    ]

    in_maps = []
    for c in range(NCORES):
        b, s = c // 4, c % 4
        gl = c % 4
        sl = slice(QSL * s, QSL * s + QSL)
        m = {
            "Kb": Kb_bf[b],
            "Qb": np.ascontiguousarray(Q[b][:, sl]).astype(BF),
            "Qres": np.ascontiguousarray(Q[b][:, sl]),
            "wq": wq_t,
            "wk": np.ascontiguousarray(wk_t[2 * gl : 2 * gl + 2]),
            "wv": np.ascontiguousarray(wv_t[:, :, 256 * gl : 256 * gl + 256]),
            "wp": wp_t, "w1": w1_t, "w2": w2_t, "b1t": b1_t,
        }
        if bq_nz:
            m["bqt"] = np.ascontiguousarray(bq_eff.reshape(8, 128).T)
        if bk_nz:
            m["bkt"] = np.ascontiguousarray(
                bk.reshape(8, 128).T[:, 2 * gl : 2 * gl + 2]
            )
        if bp_nz:
            m["bpt"] = np.ascontiguousarray(bp_eff.reshape(8, 128).T)
        if b2_nz:
            m["b2t"] = np.ascontiguousarray(b2.reshape(8, 128).T)
        if not mask_ones:
            m["maddt"] = madd_t[b]
        in_maps.append(m)

    from concourse.bass_utils import run_bass_kernel_spmd

    res = run_bass_kernel_spmd(nc, in_maps, list(range(NCORES)))

    out = np.empty((B, DIM, SQ), np.float32)
    for c in range(NCORES):
        b, s = c // 4, c % 4
        out[b][:, QSL * s : QSL * s + QSL] = res.results[c]["out"]
    return out


# revision 16
# speedup vs baseline: 1.0056x; 1.0056x over previous
"""Trainium2 Bass kernel for the MAB dense-transformer block.

Sharding: 8 cores = 2 batches x 4 Sq-slices (512 each). The K/V projection
is additionally sharded across the 4 cores of each batch (core c computes
k/v only for head-group c%4) and exchanged with two in-group AllGathers,
eliminating the 4x replication of the baseline.

Attention per core: 16 heads x 512 q-cols, processed in two 256-col halves
so the second half's ACT(sigmoid) window overlaps the first half's
proj+FFN matmuls on the PE.
  - logits^T chunks [128k x 256q] per head, E/O head pairs run as
    concurrent 64-contraction row-groups,
  - sigmoid: one 2048-wide ACT per 512 k-positions covering both heads
    (cols = [u0E u0O u1E u1O u2E u2O u3E u3O] x 256), output fp8,
  - o^T accumulated with fp8 DoubleRow matmuls (contraction 256) with a
    ones-column in vT giving row-sums for the renormalization for free,
  - renorm: DVE reciprocal of the sums row + gpsimd partition-broadcast.
Projections / FFN in bf16 with fp32 PSUM accumulation.
"""

import numpy as np
import ml_dtypes

BF = ml_dtypes.bfloat16
F8 = ml_dtypes.float8_e4m3fn

B, DIM, H, DK, SQ, SK = 2, 1024, 16, 64, 2048, 2048
D = H * DK
NCORES = 8
QSL = SQ // 4          # 512 columns of Sq per core
W = 256                # attention/FFN column-half width
NG = 4                 # head groups (4 heads each); also kv-shard count
GH = H // NG

_nc_cache = {}
_host_cache = {}


def _build_nc(mask_ones, bq_nz, bk_nz, bp_nz, b2_nz):
    from concourse import bacc, mybir
    import concourse.tile as tile

    bf16 = mybir.dt.bfloat16
    f32 = mybir.dt.float32
    fp8 = mybir.dt.float8e4
    AF = mybir.ActivationFunctionType
    DR = mybir.MatmulPerfMode.DoubleRow

    nc = bacc.Bacc("TRN2", num_devices=NCORES)

    d_Kb = nc.declare_dram_parameter("Kb", [DIM, SK], bf16, isOutput=False)
    d_Qb = nc.declare_dram_parameter("Qb", [DIM, QSL], bf16, isOutput=False)
    d_Qres = nc.declare_dram_parameter("Qres", [DIM, QSL], f32, isOutput=False)
    d_wq = nc.declare_dram_parameter("wq", [8, 128, 8, 128], bf16, isOutput=False)
    d_wk = nc.declare_dram_parameter("wk", [2, 128, 8, 128], bf16, isOutput=False)
    d_wv = nc.declare_dram_parameter("wv", [8, 128, GH * DK], bf16, isOutput=False)
    d_wp = nc.declare_dram_parameter("wp", [8, 128, 8, 128], bf16, isOutput=False)
    d_w1 = nc.declare_dram_parameter("w1", [16, 128, 8, 128], bf16, isOutput=False)
    d_w2 = nc.declare_dram_parameter("w2", [8, 128, 16, 128], bf16, isOutput=False)
    d_b1 = nc.declare_dram_parameter("b1t", [128, 16], f32, isOutput=False)
    d_bq = d_bk = d_bp = d_b2 = d_madd = None
    if bq_nz:
        d_bq = nc.declare_dram_parameter("bqt", [128, 8], f32, isOutput=False)
    if bk_nz:
        d_bk = nc.declare_dram_parameter("bkt", [128, 2], f32, isOutput=False)
    if bp_nz:
        d_bp = nc.declare_dram_parameter("bpt", [128, 8], f32, isOutput=False)
    if b2_nz:
        d_b2 = nc.declare_dram_parameter("b2t", [128, 8], f32, isOutput=False)
    if not mask_ones:
        d_madd = nc.declare_dram_parameter("maddt", [128, 16], f32, isOutput=False)
    d_out = nc.declare_dram_parameter("out", [DIM, QSL], f32, isOutput=True)

    VTW = 80  # padded per-head vT row (64 dims + ones col + pad for DR stride)

    with tile.TileContext(nc) as tc:
        with (
            tc.tile_pool(name="pin", bufs=1) as pin,
            tc.tile_pool(name="pw", bufs=4) as pw,
            tc.tile_pool(name="pkv", bufs=1) as pkv,
            tc.tile_pool(name="pq", bufs=1) as pq,
            tc.tile_pool(name="pwt", bufs=3) as pwt,
            tc.tile_pool(name="po", bufs=1) as po,
            tc.tile_pool(name="ph", bufs=1) as ph,
            tc.tile_pool(name="psmall", bufs=3) as psmall,
            tc.tile_pool(name="pconst", bufs=1) as pconst,
            tc.tile_pool(name="pout", bufs=2) as pout,
            tc.tile_pool(name="ppsL", bufs=1, space="PSUM") as ppsL,
            tc.tile_pool(name="ppsO", bufs=2, space="PSUM") as ppsO,
            tc.tile_pool(name="ppsP", bufs=2, space="PSUM") as ppsP,
            tc.tile_pool(name="pdram", bufs=1, space="DRAM") as pdram,
        ):
            # ---- input loads ----
            kb = pin.tile([128, 8, SK], bf16, tag="kb")
            kbr = d_Kb[:].rearrange("(c p) s -> p c s", p=128)
            for c in range(8):
                nc.sync.dma_start(out=kb[:, c, :], in_=kbr[:, c, :])
            wv_sb = pin.tile([128, 8, GH * DK], bf16, tag="wv")
            for c in range(8):
                nc.sync.dma_start(out=wv_sb[:, c, :], in_=d_wv[c])
            qb = pin.tile([128, 8, QSL], bf16, tag="qb")
            qbr = d_Qb[:].rearrange("(c p) s -> p c s", p=128)
            for c in range(8):
                nc.sync.dma_start(out=qb[:, c, :], in_=qbr[:, c, :])

            b1_sb = pconst.tile([128, 16], f32, tag="b1")
            nc.sync.dma_start(out=b1_sb, in_=d_b1[:])
            bq_sb = bk_sb = bp_sb = b2_sb = madd_sb = None
            if bq_nz:
                bq_sb = pconst.tile([128, 8], f32, tag="bq")
                nc.sync.dma_start(out=bq_sb, in_=d_bq[:])
            if bk_nz:
                bk_sb = pconst.tile([128, 2], f32, tag="bk")
                nc.sync.dma_start(out=bk_sb, in_=d_bk[:])
            if bp_nz:
                bp_sb = pconst.tile([128, 8], f32, tag="bp")
                nc.sync.dma_start(out=bp_sb, in_=d_bp[:])
            if b2_nz:
                b2_sb = pconst.tile([128, 8], f32, tag="b2")
                nc.sync.dma_start(out=b2_sb, in_=d_b2[:])
            if not mask_ones:
                madd_sb = pconst.tile([128, 16], f32, tag="madd")
                nc.sync.dma_start(out=madd_sb, in_=d_madd[:])

            # ---- local k/v projection (own head-group only) ----
            kg_loc = pkv.tile([128, 2, SK], bf16, tag="kgl")
            for lm in range(2):
                wt = pw.tile([128, 8, 128], bf16, tag="w")
                nc.sync.dma_start(out=wt, in_=d_wk[lm])
                for n in range(4):
                    ps = ppsP.tile([128, 512], f32, tag="pp")
                    for c in range(8):
                        nc.tensor.matmul(
                            ps, wt[:, c, :], kb[:, c, 512 * n : 512 * n + 512],
                            start=(c == 0), stop=(c == 7),
                        )
                    if bk_nz:
                        nc.scalar.activation(
                            kg_loc[:, lm, 512 * n : 512 * n + 512], ps, AF.Identity,
                            bias=bk_sb[:, lm : lm + 1],
                        )
                    else:
                        nc.vector.tensor_copy(
                            kg_loc[:, lm, 512 * n : 512 * n + 512], ps
                        )

            # ---- kick the k AllGather before projecting v, so the wire time
            # hides under the v projection + early q projection ----
            rg = [[0, 1, 2, 3], [4, 5, 6, 7]]
            cc_k_in = pdram.tile([2 * 128, SK], bf16, tag="cki")
            cc_v_in = pdram.tile([128, 8 * 2 * GH * VTW], fp8, tag="cvi")
            cc_k_out = pdram.tile([8 * 128, SK], bf16, tag="cko")
            cc_v_out = pdram.tile([4 * 128, 8 * 2 * GH * VTW], fp8, tag="cvo")
            nc.gpsimd.dma_start(
                out=cc_k_in[:].rearrange("(m p) s -> p m s", p=128), in_=kg_loc
            )
            nc.gpsimd.collective_compute(
                "AllGather", mybir.AluOpType.bypass, replica_groups=rg,
                ins=[cc_k_in[:].opt()], outs=[cc_k_out[:].opt()],
            )

            vt_loc = pkv.tile([128, 8, 2, GH, VTW], fp8, tag="vtl")
            nc.vector.memset(vt_loc.rearrange("p tp ko h d -> p (tp ko h d)"), 1.0)
            for t in range(16):
                tp, ko = t // 2, t % 2
                ps = ppsP.tile([128, 512], f32, tag="pp")
                for c in range(8):
                    nc.tensor.matmul(
                        ps[:, 0 : GH * DK],
                        kb[:, c, 128 * t : 128 * t + 128],
                        wv_sb[:, c, :],
                        start=(c == 0), stop=(c == 7),
                    )
                nc.vector.tensor_copy(
                    vt_loc[:, tp, ko, :, 0:DK],
                    ps[:, 0 : GH * DK].rearrange("p (h d) -> p h d", h=GH),
                )
            nc.gpsimd.dma_start(
                out=cc_v_in[:],
                in_=vt_loc.rearrange("p tp ko h d -> p (tp ko h d)"),
            )
            nc.gpsimd.collective_compute(
                "AllGather", mybir.AluOpType.bypass, replica_groups=rg,
                ins=[cc_v_in[:].opt()], outs=[cc_v_out[:].opt()],
            )

            # ---- q projection: chunks 0-3 here (overlap the collectives),
            # chunks 4-7 interleaved into the first attention half as PE
            # filler to keep the tensor engine dense (HAM warm) ----
            q_sb = pq.tile([128, 8, QSL], bf16, tag="q")

            def qproj_unit(m):
                wt = pw.tile([128, 8, 128], bf16, tag="w")
                nc.sync.dma_start(out=wt, in_=d_wq[m])
                ps = ppsP.tile([128, 512], f32, tag="pp")
                for c in range(8):
                    nc.tensor.matmul(
                        ps, wt[:, c, :], qb[:, c, :],
                        start=(c == 0), stop=(c == 7),
                    )
                if bq_nz:
                    nc.scalar.activation(
                        q_sb[:, m, :], ps, AF.Identity, bias=bq_sb[:, m : m + 1]
                    )
                else:
                    nc.vector.tensor_copy(q_sb[:, m, :], ps)

            for m in range(4):
                qproj_unit(m)

            # ---- gather k/v back into SBUF (full, all 16 heads) ----
            kg = pq.tile([128, 8, SK], bf16, tag="kg")
            ckr = cc_k_out[:].rearrange("(m p) s -> p m s", p=128)
            for m in range(8):
                nc.sync.dma_start(out=kg[:, m, :], in_=ckr[:, m, :])
            vt = pq.tile([128, 8, 2, H, VTW], fp8, tag="vt")
            for r in range(4):
                nc.sync.dma_start(
                    out=vt[:, :, :, GH * r : GH * r + GH, :],
                    in_=cc_v_out[128 * r : 128 * r + 128, :].rearrange(
                        "p (tp ko h d) -> p tp ko h d", tp=8, ko=2, h=GH
                    ),
                )

            # ---- resident wp weights (used in both halves) ----
            wp_sb = pin.tile([128, 8, 8, 128], bf16, tag="wp")
            for m in range(8):
                nc.sync.dma_start(out=wp_sb[:, m, :, :], in_=d_wp[m])

            o_sb = po.tile([128, 8, QSL], bf16, tag="o")
            o_ff = po.tile([128, 8, QSL], bf16, tag="off")

            def emit_head_pair(g, j, hw):
                """Attention for heads (4g+2j, 4g+2j+1), q-cols [hw*W, hw*W+W).
                Per 512 k-positions: one [128,2048] logits psum (cols =
                [u0E u0O u1E u1O u2E u2O u3E u3O] x W), one 2048-wide sigmoid
                ACT into fp8, then 4 DoubleRow o-matmuls (trailing one block
                to keep the ACT latency off the PE path)."""
                oc = 2 * g + j
                ps_oEO = [
                    ppsO.tile([65, W], f32, tag="oacc", name="ps_oE"),
                    ppsO.tile([65, W], f32, tag="oacc", name="ps_oO"),
                ]

                def emit_omms(wts, kp):
                    for eo in range(2):
                        wr = wts[eo].rearrange("p (jj ko w) -> p jj ko w", jj=2, ko=2)
                        for jj in range(2):
                            nc.tensor.matmul(
                                ps_oEO[eo],
                                vt[:, 2 * kp + jj, :, 2 * oc + eo, 0:65],
                                wr[:, jj, :, :],
                                start=(kp == 0 and jj == 0),
                                stop=(kp == 3 and jj == 1),
                                perf_mode=DR,
                            )

                prev = None
                for kp in range(4):
                    ps_lE = ppsL.tile([128, 1024], f32, tag="lgE")
                    ps_lO = ppsL.tile([128, 1024], f32, tag="lgO")
                    for u in range(4):
                        t = 4 * kp + u
                        for eo, ps_l in ((0, ps_lE), (1, ps_lO)):
                            nc.tensor.matmul(
                                ps_l[:, u * W : (u + 1) * W],
                                kg[64 * eo : 64 * eo + 64, oc, 128 * t : 128 * t + 128],
                                q_sb[64 * eo : 64 * eo + 64, oc, hw * W : hw * W + W],
                                start=True, stop=True,
                            )
                    wtE = pwt.tile([128, 1024], fp8, tag="wtE")
                    wtO = pwt.tile([128, 1024], fp8, tag="wtO")
                    if mask_ones:
                        nc.scalar.activation(wtE, ps_lE, AF.Sigmoid)
                        nc.scalar.activation(wtO, ps_lO, AF.Sigmoid)
                    else:
                        for u in range(4):
                            t = 4 * kp + u
                            for wt_t, ps_l in ((wtE, ps_lE), (wtO, ps_lO)):
                                sl = slice(u * W, (u + 1) * W)
                                nc.scalar.activation(
                                    wt_t[:, sl], ps_l[:, sl], AF.Sigmoid,
                                    bias=madd_sb[:, t : t + 1],
                                )
                    if prev is not None:
                        emit_omms(*prev)
                    prev = ((wtE, wtO), kp)
                emit_omms(*prev)

                for eo in range(2):
                    sc = psmall.tile([1, W], f32, tag="sc")
                    nc.vector.tensor_copy(sc, ps_oEO[eo][64:65, :])
                    rc = psmall.tile([1, W], f32, tag="rc")
                    nc.vector.reciprocal_approx_fast(out=rc, in_=sc)
                    rb = psmall.tile([64, W], f32, tag="rb")
                    nc.gpsimd.partition_broadcast(rb, rc)
                    nc.vector.tensor_mul(
                        o_sb[64 * eo : 64 * eo + 64, oc, hw * W : hw * W + W],
                        ps_oEO[eo][0:64, :], rb,
                    )

            def emit_proj_ffn(hw, units_only=False):
                """wp + residual + FFN for q-cols [hw*W, hw*W+W). Returns a
                list of emission units so the caller can interleave them."""
                units = []
                cs = slice(hw * W, hw * W + W)

                def proj_unit(m):
                    ps = ppsP.tile([128, 512], f32, tag="pp")
                    for c in range(8):
                        nc.tensor.matmul(
                            ps[:, 0:W], wp_sb[:, m, c, :], o_sb[:, c, cs],
                            start=(c == 0), stop=(c == 7),
                        )
                    if bp_nz:
                        nc.scalar.activation(
                            ps[:, 0:W], ps[:, 0:W], AF.Identity,
                            bias=bp_sb[:, m : m + 1],
                        )
                    qr = psmall.tile([128, W], f32, tag="qr")
                    nc.sync.dma_start(
                        out=qr, in_=d_Qres[128 * m : 128 * m + 128, cs]
                    )
                    nc.vector.tensor_add(o_ff[:, m, cs], ps[:, 0:W], qr)

                h_sb = ph.tile([128, 16, W], bf16, tag="h", bufs=2)

                def ffn1_unit(mm):
                    wt = pw.tile([128, 8, 128], bf16, tag="w")
                    nc.sync.dma_start(out=wt, in_=d_w1[mm])
                    ps = ppsP.tile([128, 512], f32, tag="pp")
                    for c in range(8):
                        nc.tensor.matmul(
                            ps[:, 0:W], wt[:, c, :], o_ff[:, c, cs],
                            start=(c == 0), stop=(c == 7),
                        )
                    nc.scalar.activation(
                        h_sb[:, mm, :], ps[:, 0:W], AF.Relu, bias=b1_sb[:, mm : mm + 1]
                    )

                def ffn2_unit(m):
                    wt = pw.tile([128, 16, 128], bf16, tag="w2", bufs=2)
                    nc.sync.dma_start(out=wt, in_=d_w2[m])
                    ps = ppsP.tile([128, 512], f32, tag="pp")
                    for c in range(16):
                        nc.tensor.matmul(
                            ps[:, 0:W], wt[:, c, :], h_sb[:, c, :],
                            start=(c == 0), stop=(c == 15),
                        )
                    if b2_nz:
                        nc.scalar.activation(
                            ps[:, 0:W], ps[:, 0:W], AF.Identity,
                            bias=b2_sb[:, m : m + 1],
                        )
                    ot = pout.tile([128, W], f32, tag="out")
                    nc.vector.tensor_add(ot, ps[:, 0:W], o_ff[:, m, cs])
                    nc.sync.dma_start(
                        out=d_out[128 * m : 128 * m + 128, cs], in_=ot
                    )

                for m in range(8):
                    units.append(lambda m=m: proj_unit(m))
                for mm in range(16):
                    units.append(lambda mm=mm: ffn1_unit(mm))
                for m in range(8):
                    units.append(lambda m=m: ffn2_unit(m))
                return units

            # half 0: attention with the remaining q-projection chunks as
            # PE filler (pair p consumes q chunk p, so chunk 4+i lands
            # after pair i)
            pi = 0
            for g in range(NG):
                for j in range(2):
                    emit_head_pair(g, j, 0)
                    if pi < 4:
                        qproj_unit(4 + pi)
                        pi += 1
            # half 1: attention with half-0 proj+FFN interleaved into the
            # sigmoid-bound window (4 single-chunk units per pair)
            units0 = emit_proj_ffn(0)
            ui = 0
            for g in range(NG):
                for j in range(2):
                    emit_head_pair(g, j, 1)
                    for _ in range(4):
                        if ui < len(units0):
                            units0[ui]()
                            ui += 1
            while ui < len(units0):
                units0[ui]()
                ui += 1
            for u in emit_proj_ffn(1):
                u()

    nc.finalize()
    return nc


def _tile_lhsT(wT, mt, ct):
    # wT [K, M] -> [M/128, 128, K/128, 128] tiles: [m, p, c, j] = wT[128c+p, 128m+j]
    K, M = wT.shape
    a = wT.reshape(K // 128, 128, M // 128, 128)
    return np.ascontiguousarray(a.transpose(2, 1, 0, 3))


def kernel(**inputs):
    np32 = lambda x: np.asarray(x, dtype=np.float32)
    Q = np32(inputs["Q"]); K = np32(inputs["K"]); mask = np32(inputs["mask"])
    wq = np32(inputs["wq"]); bq = np32(inputs["bq"])
    wk = np32(inputs["wk"]); bk = np32(inputs["bk"])
    wv = np32(inputs["wv"]); bv = np32(inputs["bv"])
    wp = np32(inputs["wp"]); bp = np32(inputs["bp"])
    w1 = np32(inputs["w1"]); b1 = np32(inputs["b1"])
    w2 = np32(inputs["w2"]); b2 = np32(inputs["b2"])

    scale = DK ** -0.5
    wq_eff = wq * scale
    bq_eff = bq * scale
    bp_eff = bp + wp @ bv          # fold v bias through the projection

    mask_ones = bool(np.all(mask == 1.0))
    bq_nz = bool(np.any(bq_eff)); bk_nz = bool(np.any(bk))
    bp_nz = bool(np.any(bp_eff)); b2_nz = bool(np.any(b2))

    key = (mask_ones, bq_nz, bk_nz, bp_nz, b2_nz)
    if key not in _nc_cache:
        _nc_cache[key] = _build_nc(*key)
    nc = _nc_cache[key]

    wkey = tuple(
        (a.__array_interface__["data"][0], a.shape)
        for a in (wq, wk, wv, wp, w1, w2, b1)
    )
    cached = _host_cache.get("w")
    if cached is not None and cached[0] == wkey:
        wq_t, wk_t, wv_t, wp_t, w1_t, w2_t, b1_t = cached[1]
    else:
        wq_t = _tile_lhsT(wq_eff.T, 8, 8).astype(BF)
        wk_t = _tile_lhsT(wk.T, 8, 8).astype(BF)
        wv_t = np.ascontiguousarray(wv.T.reshape(8, 128, D)).astype(BF)
        wp_t = _tile_lhsT(wp.T, 8, 8).astype(BF)
        w1_t = _tile_lhsT(w1.T, 16, 8).astype(BF)
        w2_t = _tile_lhsT(w2.T, 8, 16).astype(BF)
        b1_t = np.ascontiguousarray(b1.reshape(16, 128).T)
        _host_cache["w"] = (wkey, (wq_t, wk_t, wv_t, wp_t, w1_t, w2_t, b1_t))

    Kb_bf = [np.ascontiguousarray(K[b]).astype(BF) for b in range(B)]
    madd_t = [
        np.ascontiguousarray((-(1.0 - mask[b, 0]) * 10000.0).reshape(16, 128).T)
        for b in range(B)
    ]

    in_maps = []
    for c in range(NCORES):
        b, s = c // 4, c % 4
        gl = c % 4
        sl = slice(QSL * s, QSL * s + QSL)
        m = {
            "Kb": Kb_bf[b],
            "Qb": np.ascontiguousarray(Q[b][:, sl]).astype(BF),
            "Qres": np.ascontiguousarray(Q[b][:, sl]),
            "wq": wq_t,
            "wk": np.ascontiguousarray(wk_t[2 * gl : 2 * gl + 2]),
            "wv": np.ascontiguousarray(wv_t[:, :, 256 * gl : 256 * gl + 256]),
            "wp": wp_t, "w1": w1_t, "w2": w2_t, "b1t": b1_t,
        }
        if bq_nz:
            m["bqt"] = np.ascontiguousarray(bq_eff.reshape(8, 128).T)
        if bk_nz:
            m["bkt"] = np.ascontiguousarray(
                bk.reshape(8, 128).T[:, 2 * gl : 2 * gl + 2]
            )
        if bp_nz:
            m["bpt"] = np.ascontiguousarray(bp_eff.reshape(8, 128).T)
        if b2_nz:
            m["b2t"] = np.ascontiguousarray(b2.reshape(8, 128).T)
        if not mask_ones:
            m["maddt"] = madd_t[b]
        in_maps.append(m)

    from concourse.bass_utils import run_bass_kernel_spmd

    res = run_bass_kernel_spmd(nc, in_maps, list(range(NCORES)))

    out = np.empty((B, DIM, SQ), np.float32)
    for c in range(NCORES):
        b, s = c // 4, c % 4
        out[b][:, QSL * s : QSL * s + QSL] = res.results[c]["out"]
    return out
